# revision 5
# baseline (speedup 1.0000x reference)
"""BertSelfAttention Trainium2 kernel.

Full inputs in, full output out. Sharding: 8 cores = (batch b in {0,1}) x
(head-group hg in {0..3}); each core computes 4 heads of one batch and
produces the output feature slice out[b, :, hg*256:(hg+1)*256].

Per-core device program (all cores run the same NEFF, SPMD):
  xT [1024, 2048]      hidden_states[b].T
  QT/KT computed transposed [d, s] (fp32r matmuls), stored fp16 with bias
  V computed [s, d] fp16, rows scaled by exp(mask), plus a per-head
    ones*exp(mask) column so the ctx matmul also yields softmax row sums
  scoresT [k, q] tiles via row-tiled fp16 matmuls (2 heads concurrently)
  exp on ACT directly from PSUM (scale=1/8, bias=-4 folded in)
  ctx[q, d] = expT.T @ [V|em] accumulated over 16 k-tiles, then
    per-partition normalize (reciprocal of row sum) + V-bias add on DVE.
"""

import numpy as np

B = 2
S = 2048
H = 1024
NH = 16
HD = 64

NCORES = 8
HPC = 4          # heads per core
DS = HPC * HD    # 256 output dims per core
FT = H // 128    # 8 f-tiles (contraction tiles for projections)
KT = S // 128    # 16 key tiles
ST = S // 128    # 16 s-tiles of V
QB = 4           # q blocks of 512
QBS = 512
VW = HPC * (HD + 1)  # 260: V columns + one em column per head

EXP_BIAS = -4.0  # uniform shift inside exp; cancels in softmax, guards fp16

_CACHE = {}


def _build_program():
    import concourse.bass as bass
    import concourse.mybir as mybir
    import concourse.tile as tile
    from concourse.vector_clock import ScopedClock

    f32 = mybir.dt.float32
    f32r = mybir.dt.float32r
    f16 = mybir.dt.float16
    AF = mybir.ActivationFunctionType
    OP = mybir.AluOpType

    class SplitDrainTileContext(tile.TileContext):
        """The walrus build here rejects a Drain with more than one sync
        wait ("Too many sync wait commands"); split the final drain's waits
        across multiple Drain instructions."""

        MAX_WAITS_PER_DRAIN = 1

        def _drain_and_barrier(self, tick_clock, wait_clock):
            super()._drain_and_barrier(tick_clock, wait_clock)
            self._split_multi_waits()

        def _split_multi_waits(self):
            """The walrus build here accepts at most one sync wait per
            instruction; hoist excess waits onto preceding same-engine
            NOPs."""
            k = self.MAX_WAITS_PER_DRAIN
            nc = self.nc
            for bb in nc.bb_map.values():
                il = bb.bb.instructions
                new = []
                for inst in il:
                    si = getattr(inst, "sync_info", None)
                    waits = list(si.on_wait) if si is not None and si.on_wait else []
                    if len(waits) > k:
                        for j in range(0, len(waits) - k, k):
                            nop = mybir.InstNoOp(
                                name=nc.get_next_instruction_name(),
                                engine=inst.engine,
                                sync_info=mybir.SyncInfo(
                                    on_wait=waits[j : j + k], on_update=[]
                                ),
                                bass_nofuse=True,
                            )
                            new.append(nop)
                        inst.sync_info = mybir.SyncInfo(
                            on_wait=waits[len(waits) - k :],
                            on_update=list(si.on_update) if si.on_update else [],
                        )
                    new.append(inst)
                il[:] = new

    nc = bass.Bass("TRN2", target_bir_lowering=False, debug=False,
                   num_devices=NCORES)

    xT_d = nc.dram_tensor("xT", [H, S], f32, kind="ExternalInput")
    wqT_d = nc.dram_tensor("wqT", [H, DS], f32, kind="ExternalInput")
    wkT_d = nc.dram_tensor("wkT", [H, DS], f32, kind="ExternalInput")
    wvT_d = nc.dram_tensor("wvT", [H, VW], f32, kind="ExternalInput")
    bq_d = nc.dram_tensor("bq", [2, 128, 1], f32, kind="ExternalInput")
    bk_d = nc.dram_tensor("bk", [2, 128, 1], f32, kind="ExternalInput")
    bvb_d = nc.dram_tensor("bvb", [128, DS], f32, kind="ExternalInput")
    em_d = nc.dram_tensor("em", [128, KT], f32, kind="ExternalInput")
    out_d = nc.dram_tensor("out", [S, DS], f32, kind="ExternalOutput")

    with SplitDrainTileContext(nc) as tc:
        from contextlib import ExitStack

        with ExitStack() as ctx:
            const = ctx.enter_context(tc.tile_pool(name="const", bufs=1))
            qk = ctx.enter_context(tc.tile_pool(name="qk", bufs=1))
            vp = ctx.enter_context(tc.tile_pool(name="vp", bufs=1))
            epool = ctx.enter_context(tc.tile_pool(name="epool", bufs=1))
            opool = ctx.enter_context(tc.tile_pool(name="opool", bufs=1))
            rpool = ctx.enter_context(tc.tile_pool(name="rpool", bufs=1))

            # ---- constants ----
            bq_sb = [const.tile([128, 1], f32, tag=f"bq{m}", bufs=1, name=f"bq_sb{m}") for m in range(2)]
            bk_sb = [const.tile([128, 1], f32, tag=f"bk{m}", bufs=1, name=f"bk_sb{m}") for m in range(2)]
            for m in range(2):
                nc.sync.dma_start(bq_sb[m][:], bq_d.ap()[m])
                nc.sync.dma_start(bk_sb[m][:], bk_d.ap()[m])
            bvb_sb = const.tile([128, DS], f32, tag="bvb", bufs=1, name="bvb_sb")
            nc.sync.dma_start(bvb_sb[:], bvb_d.ap())
            em_sb = const.tile([128, KT], f32, tag="em", bufs=1, name="em_sb")
            nc.sync.dma_start(em_sb[:], em_d.ap())
            ebias = const.tile([128, 1], f32, tag="ebias", bufs=1, name="ebias")
            nc.vector.memset(ebias[:], EXP_BIAS)

            # ---- persistent activations ----
            qt = [qk.tile([128, S], f16, tag=f"qt{m}", bufs=1, name=f"qt{m}") for m in range(2)]
            kt_sb = [qk.tile([128, S], f16, tag=f"kt{m}", bufs=1, name=f"kt{m}") for m in range(2)]
            vones = [vp.tile([128, VW], f16, tag=f"v{st}", bufs=1, name=f"vones{st}") for st in range(ST)]

            # ---- projection phase (scoped SBUF for xT/weights, PSUM pools) --
            with ExitStack() as pctx:
                xw = pctx.enter_context(tc.tile_pool(name="xw", bufs=1))
                xt = [xw.tile([128, S], f32, tag=f"xt{ft}", bufs=1, name=f"xt{ft}") for ft in range(FT)]
                wq_sb = [xw.tile([128, DS], f32, tag=f"wq{ft}", bufs=1, name=f"wq{ft}") for ft in range(FT)]
                wk_sb = [xw.tile([128, DS], f32, tag=f"wk{ft}", bufs=1, name=f"wk{ft}") for ft in range(FT)]
                wv_sb = [xw.tile([128, VW], f32, tag=f"wv{ft}", bufs=1, name=f"wv{ft}") for ft in range(FT)]
                for ft in range(FT):
                    fs = slice(ft * 128, (ft + 1) * 128)
                    nc.sync.dma_start(
                        xt[ft][:].bitcast(f32r), xT_d.ap()[fs, :].bitcast(f32r))
                    nc.sync.dma_start(
                        wq_sb[ft][:].bitcast(f32r), wqT_d.ap()[fs, :].bitcast(f32r))
                    nc.sync.dma_start(
                        wk_sb[ft][:].bitcast(f32r), wkT_d.ap()[fs, :].bitcast(f32r))
                    nc.sync.dma_start(
                        wv_sb[ft][:].bitcast(f32r), wvT_d.ap()[fs, :].bitcast(f32r))

                ps_qk = pctx.enter_context(
                    tc.tile_pool(name="ps_qk", bufs=4, space="PSUM"))
                ps_v = pctx.enter_context(
                    tc.tile_pool(name="ps_v", bufs=4, space="PSUM"))

                def qk_proj(w_sb, bias_sb, dst, m):
                    for nb in range(QB):
                        ns = slice(nb * QBS, (nb + 1) * QBS)
                        ps = ps_qk.tile([128, QBS], f32, tag="psqk", name="psqk")
                        for ft in range(FT):
                            nc.tensor.matmul(
                                ps[:],
                                w_sb[ft][:, m * 128:(m + 1) * 128].bitcast(f32r),
                                xt[ft][:, ns].bitcast(f32r),
                                start=(ft == 0), stop=(ft == FT - 1),
                            )
                        nc.vector.tensor_scalar_add(dst[:, ns], ps[:], bias_sb[:])

                # m=0 first so attention on head-pair 0 can start early
                qk_proj(wq_sb, bq_sb[0], qt[0], 0)
                qk_proj(wk_sb, bk_sb[0], kt_sb[0], 0)

                # V projection
                for st in range(ST):
                    ss = slice(st * 128, (st + 1) * 128)
                    ps = ps_v.tile([128, VW], f32, tag="psv", name="psv")
                    for ft in range(FT):
                        nc.tensor.matmul(
                            ps[:],
                            xt[ft][:, ss].bitcast(f32r),
                            wv_sb[ft][:].bitcast(f32r),
                            start=(ft == 0), stop=(ft == FT - 1),
                        )
                    nc.vector.tensor_scalar_mul(
                        vones[st][:], ps[:], em_sb[:, st:st + 1])
                    for hh in range(HPC):
                        c = hh * (HD + 1) + HD
                        nc.vector.tensor_copy(
                            vones[st][:, c:c + 1], em_sb[:, st:st + 1])

                qk_proj(wq_sb, bq_sb[1], qt[1], 1)
                qk_proj(wk_sb, bk_sb[1], kt_sb[1], 1)

            # ---- attention phase ----
            with ExitStack() as actx:
                ps_sc = actx.enter_context(
                    tc.tile_pool(name="ps_sc", bufs=2, space="PSUM"))
                ps_cx = actx.enter_context(
                    tc.tile_pool(name="ps_cx", bufs=2, space="PSUM"))

                BATCHES = [(0, 3), (3, 3), (6, 3), (9, 3), (12, 3), (15, 1)]

                def scores_exp(hp, qb, eA, eB):
                    qs = slice(qb * QBS, (qb + 1) * QBS)
                    for (k0, nk) in BATCHES:
                        psA = ps_sc.tile([128, 3 * QBS], f32, tag="sc", name="psc")
                        psB = ps_sc.tile([128, 3 * QBS], f32, tag="sc", name="psc")
                        for j in range(nk):
                            ktile = k0 + j
                            ks = slice(ktile * 128, (ktile + 1) * 128)
                            js = slice(j * QBS, (j + 1) * QBS)
                            nc.tensor.matmul(
                                psA[:, js],
                                kt_sb[hp][0:64, ks], qt[hp][0:64, qs],
                                tile_position=(0, 0),
                            )
                            nc.tensor.matmul(
                                psB[:, js],
                                kt_sb[hp][64:128, ks], qt[hp][64:128, qs],
                                tile_position=(64, 0),
                            )
                        w = nk * QBS
                        es = slice(k0 * QBS, k0 * QBS + w)
                        nc.scalar.activation(
                            eA[:, es], psA[:, 0:w], AF.Exp,
                            bias=ebias[:], scale=0.125)
                        nc.scalar.activation(
                            eB[:, es], psB[:, 0:w], AF.Exp,
                            bias=ebias[:], scale=0.125)

                def ctx_block(hp, qb, eA, eB):
                    qtile0 = qb * 4
                    for qq in range(4):
                        ot = opool.tile([128, 128], f32, tag="ot", bufs=4, name="ot")
                        for a, e in ((0, eA), (1, eB)):
                            hh = 2 * hp + a
                            cps = ps_cx.tile([128, HD + 1], f32, tag="cx", name="cps")
                            for ktile in range(KT):
                                lo = ktile * QBS + qq * 128
                                nc.tensor.matmul(
                                    cps[:],
                                    e[:, lo:lo + 128],
                                    vones[ktile][:, hh * (HD + 1):(hh + 1) * (HD + 1)],
                                    start=(ktile == 0), stop=(ktile == KT - 1),
                                )
                            r = rpool.tile([128, 1], f32, tag="r", bufs=4, name="r")
                            nc.vector.reciprocal(r[:], cps[:, HD:HD + 1])
                            nc.vector.scalar_tensor_tensor(
                                ot[:, a * 64:(a + 1) * 64],
                                cps[:, 0:HD], r[:],
                                bvb_sb[:, hh * HD:(hh + 1) * HD],
                                op0=OP.mult, op1=OP.add,
                            )
                        qt_idx = qtile0 + qq
                        nc.sync.dma_start(
                            out_d.ap()[qt_idx * 128:(qt_idx + 1) * 128,
                                       hp * 128:(hp + 1) * 128],
                            ot[:],
                        )

                prev = None
                for hp in range(2):
                    for qb in range(QB):
                        eA = epool.tile([128, KT * QBS], f16, tag="eA", bufs=2, name="eA")
                        eB = epool.tile([128, KT * QBS], f16, tag="eB", bufs=2, name="eB")
                        scores_exp(hp, qb, eA, eB)
                        if prev is not None:
                            ctx_block(*prev)
                        prev = (hp, qb, eA, eB)
                ctx_block(*prev)

    return nc


def _get_program():
    if "nc" not in _CACHE:
        _CACHE["nc"] = _build_program()
    return _CACHE["nc"]


def _make_in_maps(hidden_states, attention_mask, Wq, bq, Wk, bk, Wv, bv):
    hidden = np.ascontiguousarray(np.asarray(hidden_states, dtype=np.float32))
    mask = np.asarray(attention_mask, dtype=np.float32)
    Wq = np.asarray(Wq, dtype=np.float32)
    Wk = np.asarray(Wk, dtype=np.float32)
    Wv = np.asarray(Wv, dtype=np.float32)
    bq = np.asarray(bq, dtype=np.float32)
    bk = np.asarray(bk, dtype=np.float32)
    bv = np.asarray(bv, dtype=np.float32)

    WqT = Wq.T  # [in, out]
    WkT = Wk.T
    WvT = Wv.T

    in_maps = []
    for c in range(NCORES):
        b, hg = divmod(c, HPC)
        cols = slice(hg * DS, (hg + 1) * DS)
        xT = np.ascontiguousarray(hidden[b].T)
        wqT = np.ascontiguousarray(WqT[:, cols])
        wkT = np.ascontiguousarray(WkT[:, cols])
        wv_base = WvT[:, cols]
        wvT = np.zeros((H, VW), np.float32)
        for hh in range(HPC):
            wvT[:, hh * (HD + 1):hh * (HD + 1) + HD] = \
                wv_base[:, hh * HD:(hh + 1) * HD]
        bq_c = np.ascontiguousarray(bq[cols].reshape(2, 128, 1))
        bk_c = np.ascontiguousarray(bk[cols].reshape(2, 128, 1))
        bvb = np.ascontiguousarray(np.tile(bv[cols][None, :], (128, 1)))
        em = np.ascontiguousarray(
            np.exp(mask[b, 0, 0, :]).reshape(KT, 128).T.astype(np.float32))
        in_maps.append({
            "xT": xT, "wqT": wqT, "wkT": wkT, "wvT": wvT,
            "bq": bq_c, "bk": bk_c, "bvb": bvb, "em": em,
        })
    return in_maps


def _assemble(results):
    out = np.empty((B, S, H), np.float32)
    for c in range(NCORES):
        b, hg = divmod(c, HPC)
        out[b][:, hg * DS:(hg + 1) * DS] = results[c]["out"]
    return out


def _run(in_maps, trace=False):
    from concourse.bass_utils import run_bass_kernel_spmd
    nc = _get_program()
    return run_bass_kernel_spmd(
        nc, in_maps, core_ids=list(range(NCORES)), trace=trace)


def kernel(**inputs):
    in_maps = _make_in_maps(**inputs)
    res = _run(in_maps, trace=False)
    return _assemble(res.results)
